# revision 1
# baseline (speedup 1.0000x reference)
"""Trainium2 Bass kernel for nn_ExpandingLinear.

Computation (see reference):
    x_exp = concat([x, x[:, p0] * v0, x_exp1[:, p1] * v1], axis=1)   # [B, 2176]
    W     = scatter_add(weight_vals at [weight_rows, weight_cols])    # [2048, 2176]
    b     = scatter_add(bias_vals at bias_idx)                        # [2048]
    out   = x_exp @ W.T + b                                           # [B, 2048]

Sharding: data-parallel over the batch dim across 8 NeuronCores (1024 rows
per core); the weight/bias/embed parameters are replicated.

Host-side prep is limited to sharding and parameter/layout preparation
(batch split, x transpose, COO->dense weight densification, embed parent-chain
resolution); all O(batch) compute — the embed feature construction, the
full dense matmul and the bias add — runs on device.

Device kernel (per core):
  - xt [2048, 1024] (x shard, feature-major) streamed in as [128,128] k-tiles
  - wt [2176, 2048] (W^T) resident in SBUF
  - 128 embed features built on device: indirect-DMA row gather from xt
    + per-partition scale; forms contraction k-tile 16
  - out[m*128:(m+1)*128, :] = sum_k xt_tile[k,m].T @ wt_tile[k] + bias
    (PE matmul in float32r, PSUM fp32 accumulation over 17 k-tiles)
"""

import numpy as np
from contextlib import ExitStack

OUT = 2048
IN_BASE = 2048
N_EMBED = 64
IN_TOT = IN_BASE + 2 * N_EMBED  # 2176
BATCH = 8192
N_CORES = 8
B_CORE = BATCH // N_CORES       # 1024
P = 128
K_TILES = IN_TOT // P           # 17
M_TILES = B_CORE // P           # 8
N_SPLIT = 4                     # 2048 out cols in 4 x 512 (one PSUM bank each)

_CACHED = {}


def _build_nc():
    import concourse.bass as bass
    import concourse.mybir as mybir
    import concourse.tile as tile
    from concourse import bacc
    from concourse.tile_rust import add_dep_helper

    f32 = mybir.dt.float32
    f32r = mybir.dt.float32r
    i32 = mybir.dt.int32

    nc = bacc.Bacc("TRN2", target_bir_lowering=False, debug=False,
                   num_devices=N_CORES)

    xt = nc.dram_tensor("xt", [B_CORE, (K_TILES - 1) * P], f32r,
                        kind="ExternalInput")  # pre-tiled: row m*128+p
    xg = nc.dram_tensor("xg", [IN_BASE, B_CORE], f32r, kind="ExternalInput")
    wt = nc.dram_tensor("wt", [IN_TOT, OUT], f32r, kind="ExternalInput")
    bias = nc.dram_tensor("bias", [P, OUT], f32, kind="ExternalInput")
    emb_q = nc.dram_tensor("emb_q", [P, 1], i32, kind="ExternalInput")
    emb_a = nc.dram_tensor("emb_a", [P, 1], f32, kind="ExternalInput")
    out = nc.dram_tensor("out", [B_CORE, OUT], f32, kind="ExternalOutput")

    NW = 512  # wt stream chunk width == one fp32 PSUM bank

    with tile.TileContext(nc) as tc:
        with ExitStack() as ctx:
            wt_pool = ctx.enter_context(tc.tile_pool(name="wt", bufs=18))
            xt_pool = ctx.enter_context(tc.tile_pool(name="xt", bufs=M_TILES))
            small_pool = ctx.enter_context(tc.tile_pool(name="small", bufs=1))
            out_pool = ctx.enter_context(tc.tile_pool(name="out", bufs=4))
            psum_pool = ctx.enter_context(
                tc.tile_pool(name="psum", bufs=8, space="PSUM"))

            # gpsimd SWDGE queue order matters (FIFO): embed params + first
            # xt tile + gather first, remaining xt tiles, bias last (only
            # needed at first evac). Both HWDGE queues stream wt chunks.
            q_t = small_pool.tile([P, 1], i32, tag="q")
            nc.sync.dma_start(out=q_t[:], in_=emb_q.ap())
            a_t = small_pool.tile([P, 1], f32, tag="a")
            nc.sync.dma_start(out=a_t[:], in_=emb_a.ap())

            xt_tiles = []

            # ring assignment for the startup-critical path: the first MM
            # needs xt0 AND wt chunk 0. chunk 0 goes at the HEAD of the sync
            # ring; xt0 alone at the head of the scalar ring (its chunks
            # queue behind 1.1MB only); xt1-7 stream on gpsimd at ~1 tile
            # per 3us, just ahead of PE's 3.9us-per-m-block consumption.
            xt_dmas = []

            def load_xt(m):
                xt_m = xt_pool.tile([P, (K_TILES - 1) * P], f32r, tag="xt",
                                    name=f"xt_m{m}")
                eng = nc.scalar if m == 0 else nc.gpsimd
                di = eng.dma_start(
                    out=xt_m[:], in_=xt.ap()[m * P:(m + 1) * P, :])
                xt_dmas.append(di)
                xt_tiles.append(xt_m)

            for m in range(4):
                load_xt(m)

            # embed features: gather parent rows of x (feature-major copy),
            # scale by alpha. partition j = expanded feature 2048+j; forms
            # k-tile 16 of x_exp^T. Emitted mid-xt-stream: needed by the end
            # of round 0a; its Q7-side sem wait (on q_t) is satisfied by
            # then, so it does not stall the SWDGE ring.
            emb_raw = small_pool.tile([P, B_CORE], f32r, tag="emb_raw")
            nc.gpsimd.indirect_dma_start(
                out=emb_raw[:],
                out_offset=None,
                in_=xg.ap(),
                in_offset=bass.IndirectOffsetOnAxis(ap=q_t[:, 0:1], axis=0),
            )
            emb_t = small_pool.tile([P, B_CORE], f32r, tag="emb")
            nc.vector.tensor_scalar_mul(
                emb_t[:], emb_raw[:].bitcast(f32), a_t[:, 0:1])

            for m in range(4, M_TILES):
                load_xt(m)

            bias_t = small_pool.tile([P, OUT], f32, tag="bias")
            nc.gpsimd.dma_start(out=bias_t[:], in_=bias.ap())

            # stream W^T n-major in [128, CK*512] chunks alternating across
            # both HWDGE queues; 8 single-bank PSUM accumulators = all 8
            # m-tiles in flight per n, so PE starts as soon as chunk 0 lands.
            CK = 2
            wt_ap3 = wt.ap().rearrange("(k p) n -> p k n", p=P)  # [128,17,2048]
            k_chunks = [(k0, min(CK, K_TILES - k0))
                        for k0 in range(0, K_TILES, CK)]
            dma_engines = [nc.sync, nc.scalar]
            ci = 0

            def load_wt_chunk(n, k0, klen):
                wck = wt_pool.tile([P, CK * NW], f32r, tag="wck",
                                   name=f"wck_n{n}_k{k0}")
                nc_dma = dma_engines[load_wt_chunk.ci % 2]
                load_wt_chunk.ci += 1
                di = nc_dma.dma_start(
                    out=wck[:, :klen * NW].rearrange(
                        "p (k c) -> p k c", k=klen),
                    in_=wt_ap3[:, k0:k0 + klen, n * NW:(n + 1) * NW])
                if n == 1:
                    # round 1 yields the t0 window to round 0's loads, but
                    # only until xt3 lands (gating on the last xt tile can
                    # block round-0 chunks via scheduler reordering).
                    add_dep_helper(di.ins, xt_dmas[3].ins, sync=True,
                                   reason="wt r1 prefetch yields to xt0-3")
                elif n > 1:
                    # rounds 2+: yield startup bandwidth to the xt stream
                    add_dep_helper(di.ins, xt_dmas[-1].ins, sync=True,
                                   reason="wt prefetch yields to xt stream")
                return wck

            load_wt_chunk.ci = 0

            def mm(psum, k, m, wck, kk):
                if k < K_TILES - 1:
                    lhsT = xt_tiles[m][:, k * P:(k + 1) * P]
                else:
                    lhsT = emb_t[:, m * P:(m + 1) * P]
                nc.tensor.matmul(
                    psum[:],
                    lhsT=lhsT,
                    rhs=wck[:, kk * NW:(kk + 1) * NW],
                    start=(k == 0),
                    stop=(k == K_TILES - 1),
                )

            for n in range(N_SPLIT):
                psums = [psum_pool.tile([P, NW], f32, tag="ps",
                                        name=f"ps_n{n}_m{m}")
                         for m in range(M_TILES)]
                if n == 0:
                    # round 0: two half-rounds (m0-3 then m4-7), k-inner
                    # within each — a half-round needs only 4 xt tiles and
                    # consumes each wt chunk at 4 MMs/chunk, matching the
                    # chunk arrival rate; the chunks are reused by the
                    # second half from SBUF.
                    wcks = [load_wt_chunk(n, k0, klen)
                            for k0, klen in k_chunks]
                    for mg in (range(0, 4), range(4, M_TILES)):
                        for (k0, klen), wck in zip(k_chunks, wcks):
                            for kk in range(klen):
                                for m in mg:
                                    mm(psums[m], k0 + kk, m, wck, kk)
                else:
                    # rounds 1-3: chunks are fully prefetched by round
                    # start, so iterate m-outer — psum completions stagger
                    # through the round, spreading evac+store (and freeing
                    # PSUM banks for the next round progressively) instead
                    # of serializing at round end.
                    wcks = [load_wt_chunk(n, k0, klen)
                            for k0, klen in k_chunks]
                    for m in range(M_TILES):
                        for (k0, klen), wck in zip(k_chunks, wcks):
                            for kk in range(klen):
                                mm(psums[m], k0 + kk, m, wck, kk)
                for m in range(M_TILES):
                    ot = out_pool.tile([P, NW], f32, tag="ot")
                    nc.vector.tensor_add(
                        ot[:], psums[m][:], bias_t[:, n * NW:(n + 1) * NW])
                    # last round: HWDGE rings are idle once the wt stream
                    # ends — use them for the final stores (faster receipts)
                    st_eng = (dma_engines[m % 2] if n == N_SPLIT - 1
                              else nc.gpsimd)
                    st_eng.dma_start(
                        out=out.ap()[m * P:(m + 1) * P, n * NW:(n + 1) * NW],
                        in_=ot[:])

    nc.compile()
    return nc


def _host_prep(inputs):
    x = np.ascontiguousarray(np.asarray(inputs["x"], dtype=np.float32))
    wv = np.asarray(inputs["weight_vals"], dtype=np.float32)
    wr = np.asarray(inputs["weight_rows"]).astype(np.int64)
    wc = np.asarray(inputs["weight_cols"]).astype(np.int64)
    bv = np.asarray(inputs["bias_vals"], dtype=np.float32)
    bi = np.asarray(inputs["bias_idx"]).astype(np.int64)
    e0v = np.asarray(inputs["embed0_vals"], dtype=np.float32)
    e0p = np.asarray(inputs["embed0_parents"]).astype(np.int64)
    e1v = np.asarray(inputs["embed1_vals"], dtype=np.float32)
    e1p = np.asarray(inputs["embed1_parents"]).astype(np.int64)

    # dense W^T [IN_TOT, OUT] (coalesce: duplicates sum)
    wt = np.bincount(wc * OUT + wr, weights=wv,
                     minlength=IN_TOT * OUT).reshape(IN_TOT, OUT)
    wt = np.ascontiguousarray(wt.astype(np.float32))

    b = np.bincount(bi, weights=bv, minlength=OUT).astype(np.float32)
    bias_bcast = np.ascontiguousarray(
        np.broadcast_to(b[None, :], (P, OUT)).astype(np.float32))

    # resolve embed parent chains to direct (row-in-x, multiplier) pairs
    q = np.empty(2 * N_EMBED, dtype=np.int32)
    a = np.empty(2 * N_EMBED, dtype=np.float32)
    q[:N_EMBED] = e0p
    a[:N_EMBED] = e0v
    for j in range(N_EMBED):
        p = int(e1p[j])
        if p < IN_BASE:
            q[N_EMBED + j] = p
            a[N_EMBED + j] = e1v[j]
        else:
            t = p - IN_BASE
            q[N_EMBED + j] = e0p[t]
            a[N_EMBED + j] = e1v[j] * e0v[t]

    xts = []
    xgs = []
    for i in range(N_CORES):
        xs = x[i * B_CORE:(i + 1) * B_CORE]
        # SBUF-tiled layout: row m*128+p, col k*128+f  ==  xs[m*128+f, k*128+p]
        xts.append(np.ascontiguousarray(
            xs.reshape(M_TILES, P, K_TILES - 1, P)
              .transpose(0, 3, 2, 1).reshape(B_CORE, (K_TILES - 1) * P)))
        xgs.append(np.ascontiguousarray(xs.T))
    return xts, xgs, wt, bias_bcast, q.reshape(P, 1), a.reshape(P, 1)


def kernel(**inputs) -> np.ndarray:
    import time
    from concourse.bass_utils import run_bass_kernel_spmd

    if "nc" not in _CACHED:
        _CACHED["nc"] = _build_nc()
    nc = _CACHED["nc"]

    xts, xgs, wt, bias_bcast, q, a = _host_prep(inputs)
    in_maps = [
        dict(xt=xts[i], xg=xgs[i], wt=wt, bias=bias_bcast, emb_q=q, emb_a=a)
        for i in range(N_CORES)
    ]
    res = None
    last_exc = None
    for attempt in range(3):
        try:
            res = run_bass_kernel_spmd(nc, in_maps,
                                       core_ids=list(range(N_CORES)))
            break
        except Exception as e:  # transient device/runtime hiccups
            last_exc = e
            time.sleep(2.0)
    if res is None:
        raise last_exc
    out = np.concatenate([res.results[i]["out"] for i in range(N_CORES)],
                         axis=0)
    return np.ascontiguousarray(out.astype(np.float32))



# revision 7
# speedup vs baseline: 1.4865x; 1.4865x over previous
"""Trainium2 Bass kernel for nn_ExpandingLinear.

Reference computation:
    x_exp = concat([x, x[:, p0] * v0, x_exp1[:, p1] * v1], axis=1)   # [B, 2176]
    W     = scatter_add(weight_vals at [weight_rows, weight_cols])    # [2048, 2176]
    b     = scatter_add(bias_vals at bias_idx)                        # [2048]
    out   = x_exp @ W.T + b                                           # [B, 2048]

Every expanded feature c is a_c * x[:, q_c] for a resolvable (q_c, a_c)
(parent chains only reference earlier features), so the embed columns fold
into the base weight on the host:
    W_eff[o, q_c] += a_c * W[o, 2048 + c]      ->  out = x @ W_eff.T + b
which reduces the device work to a dense [1024, 2048] @ [2048, 2048]
matmul + bias per core (data-parallel batch shard, 8 cores).

Numerics: x and W_eff are cast to bf16 on the host (PSUM accumulates fp32);
measured end-to-end rel err ~4e-3 against the fp32 reference, well inside
the 2e-2 gate, and bf16 halves every DMA stream vs fp32.

Device schedule (per core):
  - wt (W_eff^T, [16 k-tiles, 128, 2048]) and xt (x^T, [16, 128, 1024])
    stream into resident SBUF tiles as 256KB k-tiles on 4 queues:
    W n-cols 0:1024 on sync/scalar (k even/odd), x on gpsimd/vector.
    W n-cols 1024:2048 and the bias queue behind those (needed later).
  - round n=0 runs k-outer / m-inner, paced by the k-ordered W/x streams;
    all 8 PSUM banks accumulate one m-tile each.
  - rounds n=1..3 run m-outer / k-inner from resident SBUF, staggering
    PSUM completion so evac (vector add bias -> bf16) + store overlap
    the next m-block's matmuls.
"""

import numpy as np
from contextlib import ExitStack

OUT = 2048
IN_BASE = 2048
N_EMBED = 64
IN_TOT = IN_BASE + 2 * N_EMBED  # 2176
BATCH = 8192
N_CORES = 8
B_CORE = BATCH // N_CORES       # 1024
P = 128
K_TILES = IN_BASE // P          # 16 (embeds folded away)
M_TILES = B_CORE // P           # 8
N_SPLIT = 4                     # 2048 out cols in 4 x 512 (one PSUM bank each)
NW = 512

_CACHED = {}


def _build_nc():
    import concourse.mybir as mybir
    import concourse.tile as tile
    from concourse import bacc

    f32 = mybir.dt.float32
    bf16 = mybir.dt.bfloat16

    nc = bacc.Bacc("TRN2", target_bir_lowering=False, debug=False,
                   num_devices=N_CORES)

    xt = nc.dram_tensor("xt", [IN_BASE, B_CORE], bf16, kind="ExternalInput")
    wt = nc.dram_tensor("wt", [IN_BASE, OUT], bf16, kind="ExternalInput")
    bias = nc.dram_tensor("bias", [1, OUT], f32, kind="ExternalInput")
    out = nc.dram_tensor("out", [B_CORE, OUT], bf16, kind="ExternalOutput")

    xt_ap = xt.ap().rearrange("(k p) b -> p k b", p=P)   # [128, 16, 1024]
    wt_ap = wt.ap().rearrange("(k p) n -> p k n", p=P)   # [128, 16, 2048]

    NHALF = OUT // 2  # 1024

    with tile.TileContext(nc) as tc:
        with ExitStack() as ctx:
            big_pool = ctx.enter_context(tc.tile_pool(name="big", bufs=1))
            out_pool = ctx.enter_context(tc.tile_pool(name="out", bufs=4))
            psum_pool = ctx.enter_context(
                tc.tile_pool(name="psum", bufs=8, space="PSUM"))

            wt_sb = big_pool.tile([P, K_TILES * OUT], bf16, tag="wt")
            xt_sb = big_pool.tile([P, K_TILES * B_CORE], bf16, tag="xt")
            bias_row = big_pool.tile([1, OUT], f32, tag="bias_row")
            bias_t = big_pool.tile([P, OUT], f32, tag="bias")

            # k-ordered streams; first tiles of each are the PE-start
            # critical path. Queue FIFO order doubles as priority: the
            # n-half-B chunks drain only after the half-A/x streams they
            # queue behind.
            nc.sync.dma_start(out=bias_row[:], in_=bias.ap())
            for k in range(K_TILES):
                eng = nc.sync if k % 2 == 0 else nc.scalar
                eng.dma_start(
                    out=wt_sb[:, k * OUT:k * OUT + NHALF],
                    in_=wt_ap[:, k, 0:NHALF])
                nc.gpsimd.dma_start(
                    out=xt_sb[:, k * B_CORE:(k + 1) * B_CORE],
                    in_=xt_ap[:, k, :])
            for k in range(K_TILES):
                eng = nc.scalar if k % 2 == 0 else nc.sync
                eng.dma_start(
                    out=wt_sb[:, k * OUT + NHALF:(k + 1) * OUT],
                    in_=wt_ap[:, k, NHALF:OUT])
            nc.gpsimd.partition_broadcast(bias_t[:], bias_row[:])

            def lhsT(k, m):
                return xt_sb[:, k * B_CORE + m * P:k * B_CORE + (m + 1) * P]

            def rhs(k, n):
                return wt_sb[:, k * OUT + n * NW:k * OUT + (n + 1) * NW]

            def evac(psum, m, n):
                ot = out_pool.tile([P, NW], bf16, tag="ot")
                nc.vector.tensor_add(
                    ot[:], psum[:], bias_t[:, n * NW:(n + 1) * NW])
                nc.gpsimd.dma_start(
                    out=out.ap()[m * P:(m + 1) * P, n * NW:(n + 1) * NW],
                    in_=ot[:])

            # round n=0: k-outer so PE consumes W/x k-tiles in arrival
            # order (~240 GB/s joint stream pace, under the ~350 measured)
            psums = [psum_pool.tile([P, NW], f32, tag="ps",
                                    name=f"ps_n0_m{m}")
                     for m in range(M_TILES)]
            for k in range(K_TILES):
                for m in range(M_TILES):
                    nc.tensor.matmul(
                        psums[m][:], lhsT=lhsT(k, m), rhs=rhs(k, 0),
                        start=(k == 0), stop=(k == K_TILES - 1))
            for m in range(M_TILES):
                evac(psums[m], m, 0)

            # rounds n=1..3: resident SBUF, m-outer staggers psum
            # completion so evac+store overlap the next m-block
            for n in range(1, N_SPLIT):
                for m in range(M_TILES):
                    ps = psum_pool.tile([P, NW], f32, tag="ps",
                                        name=f"ps_n{n}_m{m}")
                    for k in range(K_TILES):
                        nc.tensor.matmul(
                            ps[:], lhsT=lhsT(k, m), rhs=rhs(k, n),
                            start=(k == 0), stop=(k == K_TILES - 1))
                    evac(ps, m, n)

    nc.compile()
    return nc


def _host_prep(inputs):
    import ml_dtypes

    x = np.asarray(inputs["x"], dtype=np.float32)
    wv = np.asarray(inputs["weight_vals"], dtype=np.float32)
    wr = np.asarray(inputs["weight_rows"]).astype(np.int64)
    wc = np.asarray(inputs["weight_cols"]).astype(np.int64)
    bv = np.asarray(inputs["bias_vals"], dtype=np.float32)
    bi = np.asarray(inputs["bias_idx"]).astype(np.int64)
    e0v = np.asarray(inputs["embed0_vals"], dtype=np.float32)
    e0p = np.asarray(inputs["embed0_parents"]).astype(np.int64)
    e1v = np.asarray(inputs["embed1_vals"], dtype=np.float32)
    e1p = np.asarray(inputs["embed1_parents"]).astype(np.int64)

    # dense W^T [IN_TOT, OUT] (coalesce: duplicates sum)
    wt_full = np.bincount(wc * OUT + wr, weights=wv,
                          minlength=IN_TOT * OUT).reshape(IN_TOT, OUT)

    # resolve embed parent chains to (row-in-x, multiplier), then fold the
    # expanded-feature rows of W^T into their parent rows
    q = np.empty(2 * N_EMBED, dtype=np.int64)
    a = np.empty(2 * N_EMBED, dtype=np.float64)
    q[:N_EMBED] = e0p
    a[:N_EMBED] = e0v
    for j in range(N_EMBED):
        p = int(e1p[j])
        if p < IN_BASE:
            q[N_EMBED + j] = p
            a[N_EMBED + j] = e1v[j]
        else:
            t = p - IN_BASE
            q[N_EMBED + j] = e0p[t]
            a[N_EMBED + j] = e1v[j] * e0v[t]
    wt_eff = wt_full[:IN_BASE]
    np.add.at(wt_eff, q, a[:, None] * wt_full[IN_BASE:])
    wt_bf = np.ascontiguousarray(wt_eff.astype(ml_dtypes.bfloat16))

    b = np.bincount(bi, weights=bv, minlength=OUT).astype(np.float32)
    bias_row = np.ascontiguousarray(b[None, :])

    x_bf = x.astype(ml_dtypes.bfloat16)
    xts = [np.ascontiguousarray(x_bf[i * B_CORE:(i + 1) * B_CORE].T)
           for i in range(N_CORES)]
    return xts, wt_bf, bias_row


def kernel(**inputs) -> np.ndarray:
    import time
    from concourse.bass_utils import run_bass_kernel_spmd

    if "nc" not in _CACHED:
        _CACHED["nc"] = _build_nc()
    nc = _CACHED["nc"]

    xts, wt_bf, bias_row = _host_prep(inputs)
    in_maps = [dict(xt=xts[i], wt=wt_bf, bias=bias_row)
               for i in range(N_CORES)]
    res = None
    last_exc = None
    for attempt in range(3):
        try:
            res = run_bass_kernel_spmd(nc, in_maps,
                                       core_ids=list(range(N_CORES)))
            break
        except Exception as e:  # transient device/runtime hiccups
            last_exc = e
            time.sleep(2.0)
    if res is None:
        raise last_exc
    out = np.concatenate([res.results[i]["out"] for i in range(N_CORES)],
                         axis=0)
    return np.ascontiguousarray(out.astype(np.float32))
